# revision 40
# baseline (speedup 1.0000x reference)
"""Bass/Trainium2 kernel for nn_BaseAttention (B=2, S=2048, H=1024, NH=16, HD=64).

Sharding: 8 cores = 2 batches x 4 head-groups (4 heads each core).
Each core computes, for its (batch b, head-group hb):
    qkv slice -> attention over packed masked keys -> partial out-projection
and writes partial^T [H, S].  Host sums the 4 partials per batch and
transposes.

v4 design (all-fp16 data path; cost-model-guided):
  * fp16 everywhere instead of bf16: same 1 cyc/row matmul throughput and
    identical DMA bytes, but 8x less quantization error -- the error budget
    is then dominated by the Schraudolph exp tiles alone (~1.1e-2).
  * AV computed in [q, d] orientation: stationary = exp'd score tile
    [128 keys, 128 q], moving = per-(kt, head) V tile [128 keys, 64].
    Matmul time is out_free x 1 cyc, so AV drops ~2x vs the [d^T, q]
    orientation; softmax denominators come from parallel 1-column matmuls
    against a ones-column into a shared PSUM bank.  PSUM start=True zeroes
    the whole 2KB bank, so interleaved sub-bank accumulation groups carry
    start only on the bank's first matmul and rely on pending-zero for the
    rest (skip_group_check).
  * Normalization is a reciprocal of the denominator row plus a broadcast
    tensor_tensor multiply (per-partition q); the [q, d] -> [d, q]
    transpose runs on the DMA XBAR (dma_start_transpose) for the first
    head-pair (latency-tolerant) and on the PE array (identity matmul into
    a 1-bank f16 PSUM tile, tiny Act/DVE evacs) for the last pair, which
    gates the out-projection.
  * Scores in S^T layout [key_part, q_free], one [128,512] PSUM bank per
    half so the shared "pss" tag rotates 5 deep; exp halves split across
    Act (accurate exp) and DVE (Schraudolph: uint16(round(s*A + B)) bits
    ARE fp16(exp(s)); saturates to +0 for masked keys) at a 20:16 ratio.
    GPSIMD cannot access PSUM, so Act+DVE carry the whole exp wall.
  * Phase schedule: warmup matmuls hold the PE p-state ramp until the DMA
    stream lands; K-projection chases the per-ht (wk, xpT) stream with two
    V key-tiles riding along; Q half 0; attention q[0:1024]; Q half 1 +
    out-projection columns 0:1024 (output DMA overlaps the second
    attention half); attention q[1024:2048] with held-back out-projection
    rows as PE filler around the final pair's normalize/transpose; out-
    projection columns 1024:2048 streams straight to the output DMA.
  * Masked keys packed on host (KP = ceil(max_count/128)*128); 1/sqrt(HD)
    folded into wq on the host; key-padding bias fused into exp.
"""

import numpy as np

import concourse.bass as bass
import concourse.mybir as mybir
import concourse.tile as tile
from concourse import bacc
from concourse import bass_utils

B, S, H = 2, 2048, 1024
NH, HD = 16, 64
SCALE = HD ** -0.5
NCORES = 8
CPB = NCORES // B          # cores per batch = 4
NHL = NH // CPB            # local heads per core = 4
QD = NHL * HD              # local head-dim total = 256
HT = H // 128              # k-tiles over hidden dim = 8
MT = QD // 128             # partition-tiles over local head dims = 2
PO = 2                     # query halves (1024 each)
QT = 8                     # 128-query tiles per half

F32 = mybir.dt.float32
F16 = mybir.dt.float16
U16 = mybir.dt.uint16
NPF16 = np.float16

# Schraudolph exp -> fp16 bits: u16 = round(s * AEXP + BEXP), saturating at
# 0.  AEXP = 2^10/ln2 (fp16 exponent LSB is bit 10); the -60 fraction-bias
# offset minimizes the max relative error over the sawtooth.
AEXP = 1024.0 / float(np.log(2.0))
BEXP = 15.0 * 1024.0 - 60.0


def _chunks(total, size):
    out = []
    o = 0
    while o < total:
        c = min(size, total - o)
        out.append((o, c))
        o += c
    return out


# Engines for the two 512-halves of one (head, kt) exp tile.
# 'a' = Act accurate exp, 'd' = DVE Schraudolph.  GPSIMD cannot access
# PSUM on TRN2, so the exp wall is carried by Act+DVE alone; the cycle
# of 9 gives Act 20 / DVE 16 halves per 9-kt pair loop.
_EXP_TABLE = [("a", "d"), ("d", "a"), ("a", "d"),
              ("d", "a"), ("a", "d"), ("d", "a"),
              ("a", "d"), ("d", "a"), ("a", "a")]


def _exp_engines(kt, hi, last=False):
    if last:
        return ("a", "d") if hi == 0 else ("d", "a")
    return _EXP_TABLE[(kt * 2 + hi) % 9]


def build_kernel(KP):
    KT = KP // 128
    nc = bacc.Bacc("TRN2")
    ident = nc.dram_tensor("ident", [128, 128], F16, kind="ExternalInput")
    xT = nc.dram_tensor("xT", [H, S], F16, kind="ExternalInput")
    xpT = nc.dram_tensor("xpT", [H, KP], F16, kind="ExternalInput")
    wqT = nc.dram_tensor("wqT", [H, QD], F16, kind="ExternalInput")
    wkT = nc.dram_tensor("wkT", [H, QD], F16, kind="ExternalInput")
    wvT = nc.dram_tensor("wvT", [H, QD], F16, kind="ExternalInput")
    woT = nc.dram_tensor("woT", [QD, H], F16, kind="ExternalInput")
    bk = nc.dram_tensor("bk", [128, KT], F32, kind="ExternalInput")
    bk2 = nc.dram_tensor("bk2", [128, KT], F32, kind="ExternalInput")
    outT = nc.dram_tensor("outT", [H, S], F16, kind="ExternalOutput")

    with tile.TileContext(nc) as tc:
        with tile.TileContext.tile_pool(tc, name="wts", bufs=1) as wp:
            wq_sb = wp.tile([128, HT, QD], F16)
            wk_sb = wp.tile([128, HT, QD], F16)
            wv_sb = wp.tile([128, HT, QD], F16)
            wo_sb = wp.tile([128, MT, H], F16)
            bk_sb = wp.tile([128, KT], F32)
            bk2_sb = wp.tile([128, KT], F32)
            xT_sb = wp.tile([128, HT, S], F16)
            xpT_sb = wp.tile([128, HT, KP], F16)
            qT_sb = wp.tile([128, MT, S], F16)
            kT_sb = wp.tile([128, MT, KP], F16)
            va_sb = wp.tile([128, KT, NHL, 65], F16)   # V rows + ones col
            aT_sb = wp.tile([128, MT, S], F16)
            id_sb = wp.tile([128, 128], F16)

            wu_sb = wp.tile([128, 128], F16)
            nc.vector.memset(wu_sb, 0.0)

            # --- input DMA on two queues: wk slices + small tensors on the
            # scalar queue, the bulk stream (xpT, wv, wq, xT, wo) on sync.
            # The two queues land wk[ht0] and xpT[ht0] in parallel so the
            # K projection's first matmul starts ~3.7us in and chases the
            # per-ht stream.
            nc.sync.dma_start(out=wk_sb,
                              in_=wkT.ap().rearrange("(t p) d -> p t d",
                                                     p=128))
            for ht in range(HT):
                nc.sync.dma_start(out=xpT_sb[:, ht, :],
                                  in_=xpT.ap()[ht * 128:(ht + 1) * 128, :])
                if ht == 5:
                    nc.sync.dma_start(
                        out=wv_sb,
                        in_=wvT.ap().rearrange("(t p) d -> p t d", p=128))
            nc.scalar.dma_start(out=bk_sb, in_=bk.ap())
            nc.scalar.dma_start(out=bk2_sb, in_=bk2.ap())
            nc.scalar.dma_start(out=id_sb, in_=ident.ap())
            nc.sync.dma_start(out=wq_sb,
                              in_=wqT.ap().rearrange("(t p) d -> p t d",
                                                     p=128))
            for hp in range(HT // 2):
                nc.sync.dma_start(
                    out=xT_sb[:, 2 * hp:2 * hp + 2, :],
                    in_=xT.ap()[hp * 256:(hp + 1) * 256, :].rearrange(
                        "(t p) s -> p t s", p=128))
            nc.sync.dma_start(out=wo_sb,
                              in_=woT.ap().rearrange("(t p) d -> p t d",
                                                     p=128))
            nc.vector.memset(va_sb[:, :, :, 64:65], 1.0)

            evac_flip = [0]

            def evac(dst, src):
                # alternate psum evacuations between Act and DVE
                if evac_flip[0] % 2 == 0:
                    nc.scalar.copy(dst, src)
                else:
                    nc.vector.tensor_copy(dst, src)
                evac_flip[0] += 1

            # single PSUM pool, 8 banks: tag "pss" = 5 rotating [128,512]f32
            # banks (projections / scores / out-proj / PE-transpose spill),
            # tag "pav" = 2 banks (AV accumulators, K remainder chunks, V
            # projection), tag "pdn" = 1 bank (softmax denominators).
            with tile.TileContext.tile_pool(tc, name="pss", bufs=5,
                                            space="PSUM") as pss:
                def pstile(shape, tag, bufs, name, dt=F32):
                    return pss.tile(shape, dt, tag=tag, bufs=bufs, name=name)

                if True:
                    # ---- PE warmup: tiny matmuls on zeros keep the tensor
                    # engine's p-state ramp running until the first real
                    # matmul's inputs land (~4.5us), so the K projection
                    # starts at full clock.
                    wps = pstile([128, 512], "pss", 5, "ps_wu")
                    for _ in range(40):
                        nc.tensor.matmul(wps[:, 0:128], wu_sb, wu_sb,
                                         start=True, stop=True)

                    # ---- K^T projection, ht-outer so matmuls chase the DMA.
                    kchunks = []
                    for mt in range(MT):
                        for po, pw in _chunks(KP, 512):
                            if pw > 128:
                                ps = pstile([128, 512], "pss", 5,
                                            f"ps_k{mt}_{po}")
                            else:
                                ps = pstile([128, 128], "pav", 2,
                                            f"ps_k{mt}_{po}")
                            kchunks.append((mt, po, pw, ps))
                    # the first two V-projection key-tiles ride along in
                    # the K ht-loop so the PE outpaces the xpT DMA stream
                    vps = [pstile([128, QD], "pss", 5, "ps_v0"),
                           pstile([128, QD], "pdn", 1, "ps_v1")]
                    for ht in range(HT):
                        for mt, po, pw, ps in kchunks:
                            nc.tensor.matmul(
                                ps[:, 0:pw],
                                wk_sb[:, ht, mt * 128:(mt + 1) * 128],
                                xpT_sb[:, ht, po:po + pw],
                                start=(ht == 0), stop=(ht == HT - 1))
                        for st in range(2):
                            nc.tensor.matmul(
                                vps[st],
                                xpT_sb[:, ht, st * 128:(st + 1) * 128],
                                wv_sb[:, ht, :],
                                start=(ht == 0), stop=(ht == HT - 1))
                    for mt, po, pw, ps in sorted(kchunks,
                                                 key=lambda c: -c[1]):
                        evac(kT_sb[:, mt, po:po + pw], ps[:, 0:pw])
                    for st in range(2):
                        evac(va_sb[:, st, :, 0:64],
                             vps[st].rearrange("p (h d) -> p h d", h=NHL))

                    # ---- V projection, remaining key-tiles
                    for st in range(2, KT):
                        pv = pstile([128, QD], "pav", 2, "ps_v")
                        for ht in range(HT):
                            nc.tensor.matmul(
                                pv, xpT_sb[:, ht, st * 128:(st + 1) * 128],
                                wv_sb[:, ht, :],
                                start=(ht == 0), stop=(ht == HT - 1))
                        evac(va_sb[:, st, :, 0:64],
                             pv.rearrange("p (h d) -> p h d", h=NHL))

                # ---- Q^T projection for a query half
                def emit_q(po, mts=(0, 1)):
                    for mt in mts:
                        for co, cw in _chunks(1024, 512):
                            ps = pstile([128, 512], "pss", 5,
                                        f"ps_q{mt}_{po + co}")
                            for ht in range(HT):
                                nc.tensor.matmul(
                                    ps,
                                    wq_sb[:, ht, mt * 128:(mt + 1) * 128],
                                    xT_sb[:, ht, po + co:po + co + cw],
                                    start=(ht == 0), stop=(ht == HT - 1))
                            evac(qT_sb[:, mt, po + co:po + co + cw], ps)

                emit_q(0)

                # ---- attention + interleaved out-projection phases
                with tile.TileContext.tile_pool(tc, name="pex", bufs=12) as pxp, \
                     tile.TileContext.tile_pool(tc, name="an", bufs=3) as anp, \
                     tile.TileContext.tile_pool(tc, name="rc", bufs=4) as rcp, \
                     tile.TileContext.tile_pool(tc, name="stg", bufs=10) as sgp:

                    def attn_pair(pair, po):
                        """Attention for heads `pair` on queries
                        [po*1024, (po+1)*1024)."""
                        mtq = pair[0] // 2
                        q0 = po * 1024
                        pav = {}
                        pend = {h: [] for h in pair}
                        for h in pair:
                            pav[h] = pstile([128, QT, 64], "pav", 2,
                                            f"pav{h}_{po}")
                        # softmax denominators for both heads (64B bank)
                        pden = pstile([128, 2, QT], "pdn", 1,
                                      f"pdn{pair[0]}_{po}")

                        def flush_av(hi, h):
                            # start=True zeroes the whole 2KB PSUM bank, so
                            # only the very first matmul into each bank may
                            # carry it; the other interleaved accumulation
                            # groups land on pending-zero bytes (zeroed on
                            # first write).
                            pkt, ppx = pend[h].pop(0)
                            for qt in range(QT):
                                pxs = ppx[:, qt * 128:(qt + 1) * 128]
                                nc.tensor.matmul(
                                    pav[h][:, qt, :], pxs,
                                    va_sb[:, pkt, h, 0:64],
                                    start=(pkt == 0 and qt == 0),
                                    stop=(pkt == KT - 1 and qt == QT - 1),
                                    skip_group_check=True)
                                nc.tensor.matmul(
                                    pden[:, hi, qt:qt + 1], pxs,
                                    va_sb[:, pkt, h, 64:65],
                                    start=(pkt == 0 and qt == 0 and hi == 0),
                                    stop=(pkt == KT - 1 and qt == QT - 1
                                          and hi == 1),
                                    skip_group_check=True)

                        for kt in range(KT):
                            for hi, h in enumerate(pair):
                                rb = (h * HD) % 128
                                px = pxp.tile([128, 1024], F16, tag="pex",
                                              name=f"pex{h}_{po}_{kt}")
                                engs = _exp_engines(kt, hi, last=(kt == KT - 1))
                                for ci, (co, cw) in enumerate(_chunks(1024, 512)):
                                    ps = pstile([128, 512], "pss", 5,
                                                f"ps_s{h}_{po}_{kt}_{co}")
                                    nc.tensor.matmul(
                                        ps,
                                        kT_sb[rb:rb + HD, mtq,
                                              kt * 128:(kt + 1) * 128],
                                        qT_sb[rb:rb + HD, mtq,
                                              q0 + co:q0 + co + cw],
                                        start=True, stop=True)
                                    eng = engs[ci]
                                    if eng == "a":
                                        nc.scalar.activation(
                                            out=px[:, co:co + cw],
                                            in_=ps,
                                            func=mybir.ActivationFunctionType.Exp,
                                            bias=bk_sb[:, kt:kt + 1], scale=1.0)
                                    else:
                                        nc.vector.tensor_scalar(
                                            px[:, co:co + cw].bitcast(U16),
                                            ps,
                                            AEXP, bk2_sb[:, kt:kt + 1],
                                            mybir.AluOpType.mult,
                                            mybir.AluOpType.add)
                                pend[h].append((kt, px))
                            # drain the AV backlog harder near the end of the
                            # kt loop so normalize/transpose start promptly
                            max_pend = 2 if kt < KT - 2 else (KT - 1 - kt)
                            for hi, h in enumerate(pair):
                                while len(pend[h]) > max_pend:
                                    flush_av(hi, h)
                        an = anp.tile([128, QT, 128], F16, tag="an",
                                      name=f"an{pair[0]}_{po}")
                        rcb = {}
                        for hi, h in enumerate(pair):
                            while pend[h]:
                                flush_av(hi, h)
                            rc = rcp.tile([128, QT], F32, tag="rc",
                                          name=f"rc{h}_{po}")
                            nc.vector.reciprocal(rc, pden[:, hi, :])
                            r = rc.rearrange("p (q o) -> p q o", o=1)
                            rcb[h] = r.broadcast_to([128, QT, 64])
                        # normalize a_n[q, qt, d] = pav[q, qt, d] / den in
                        # qt-half groups so transposes can start early
                        for hr in (0, QT // 2):
                            for hi, h in enumerate(pair):
                                nc.vector.tensor_tensor(
                                    out=an[:, hr:hr + QT // 2,
                                           hi * 64:hi * 64 + 64],
                                    in0=pav[h][:, hr:hr + QT // 2, :],
                                    in1=rcb[h][:, hr:hr + QT // 2, :],
                                    op=mybir.AluOpType.mult)

                        # transpose [q, d] -> [d, q].  The first pair rides
                        # the DMA XBAR (latency-tolerant: a full pair of
                        # compute follows); the last pair, which gates the
                        # out-projection, uses the PE array + tiny evacs.
                        # Emission is deferred to the returned closure so the
                        # caller can slot PE filler before the PE transposes.
                        tr_tile = []

                        def finish_half(hq):
                            qts = range(hq * (QT // 2), (hq + 1) * (QT // 2))
                            if mtq == 0:
                                for qt in qts:
                                    nc.sync.dma_start_transpose(
                                        aT_sb[:, mtq,
                                              q0 + qt * 128:q0 + (qt + 1) * 128],
                                        an[:, qt, :])
                                return
                            if not tr_tile:
                                tr_tile.append(pstile([128, QT, 128], "pav", 2,
                                                      f"tr{po}", dt=F16))
                            tr = tr_tile[0]
                            for qt in qts:
                                # sub-bank writes: only the first transpose
                                # into the tr bank may carry start=True
                                nc.tensor.matmul(
                                    tr[:, qt, :], an[:, qt, :], id_sb,
                                    is_transpose=True,
                                    start=(hq == 0 and qt == qts[0]),
                                    stop=(hq == 1 and qt == qts[-1]),
                                    skip_group_check=True)
                            engs = ((nc.scalar, nc.vector, nc.scalar,
                                     nc.vector) if hq == 0 else
                                    (nc.vector, nc.scalar, nc.vector,
                                     nc.scalar))
                            for qt in qts:
                                dst = aT_sb[:, mtq,
                                            q0 + qt * 128:q0 + (qt + 1) * 128]
                                copy_half(engs[qt % 4], dst, tr[:, qt, :])

                        def finish():
                            finish_half(0)
                            finish_half(1)

                        finish.half = finish_half
                        return finish

                    def copy_half(eng, dst, src):
                        if eng is nc.scalar:
                            eng.copy(dst, src)
                        else:
                            eng.tensor_copy(dst, src)

                    def out_proj(ho, jts, eoff=0):
                        """Out-projection rows jts, columns [ho*1024, +1024).
                        PSUM evacuation runs as 512-halves on two engines so
                        the pss slots free at PE pace."""
                        q0 = ho * 1024
                        for ji, jt in enumerate(jts):
                            stg = sgp.tile([128, 1024], F16, tag="stg",
                                           name="stage")
                            e01 = ((nc.scalar, nc.vector)
                                   if (ji + eoff) % 2 == 0
                                   else (nc.vector, nc.scalar))
                            for ci, (co, cw) in enumerate(_chunks(1024, 512)):
                                pf = pstile([128, 512], "pss", 5,
                                            f"ps_f{jt}_{ho}_{co}")
                                for mt in range(MT):
                                    nc.tensor.matmul(
                                        pf,
                                        wo_sb[:, mt, jt * 128:(jt + 1) * 128],
                                        aT_sb[:, mt, q0 + co:q0 + co + cw],
                                        start=(mt == 0), stop=(mt == MT - 1))
                                copy_half(e01[ci], stg[:, co:co + cw], pf)
                            nc.sync.dma_start(
                                out=outT.ap()[jt * 128:(jt + 1) * 128,
                                              q0:q0 + 1024],
                                in_=stg)

                    attn_pair((0, 1), 0)()
                    f2 = attn_pair((2, 3), 0)
                    emit_q(1024, mts=(0,))   # PE filler while normalize lands
                    f2()
                    emit_q(1024, mts=(1,))
                    out_proj(0, range(5))
                    attn_pair((0, 1), 1)()
                    f4 = attn_pair((2, 3), 1)
                    # held-back ho=0 rows fill the PE while the last pair's
                    # normalize lands, then its PE transposes + evacs run
                    out_proj(0, range(5, 6), eoff=1)
                    f4.half(0)
                    out_proj(0, range(6, HT), eoff=1)
                    f4.half(1)
                    out_proj(1, range(HT))

    nc.compile()
    return nc


def _prep_inputs(hidden_states, attention_mask, w_qkv, w_out):
    """Shard + transpose + quantize inputs for the 8 cores."""
    hs = np.asarray(hidden_states, dtype=np.float32)
    mask = np.asarray(attention_mask)
    wqkv = np.asarray(w_qkv, dtype=np.float32)
    wo = np.asarray(w_out, dtype=np.float32)

    idxs = [np.nonzero(mask[b] != 0)[0] for b in range(B)]
    counts = [len(ix) for ix in idxs]
    KP = max(128, ((max(counts) + 127) // 128) * 128)
    KT = KP // 128

    xTs, xpTs, bks, bk2s = [], [], [], []
    for b in range(B):
        xb = hs[b].astype(NPF16)
        xTs.append(np.ascontiguousarray(xb.T))
        xp = np.zeros((KP, H), dtype=NPF16)
        xp[:counts[b]] = xb[idxs[b]]
        xpTs.append(np.ascontiguousarray(xp.T))
        bias = np.zeros(KP, dtype=np.float32)
        bias[counts[b]:] = -30000.0
        bias = np.ascontiguousarray(bias.reshape(KT, 128).T)
        bks.append(bias)
        bk2s.append(np.ascontiguousarray(
            (bias * AEXP + BEXP).astype(np.float32)))

    ident = np.ascontiguousarray(np.eye(128, dtype=NPF16))
    in_maps = []
    for c in range(NCORES):
        b, hb = c // CPB, c % CPB
        sl = slice(hb * QD, (hb + 1) * QD)
        in_maps.append({
            "ident": ident,
            "xT": xTs[b],
            "xpT": xpTs[b],
            "wqT": np.ascontiguousarray(
                (wqkv[sl, :] * SCALE).astype(NPF16).T),
            "wkT": np.ascontiguousarray(
                wqkv[H + sl.start:H + sl.stop, :].astype(NPF16).T),
            "wvT": np.ascontiguousarray(
                wqkv[2 * H + sl.start:2 * H + sl.stop, :].astype(NPF16).T),
            "woT": np.ascontiguousarray(wo[:, sl].astype(NPF16).T),
            "bk": bks[b],
            "bk2": bk2s[b],
        })
    return KP, in_maps


_NC_CACHE = {}


def kernel(hidden_states, attention_mask, w_qkv, w_out):
    KP, in_maps = _prep_inputs(hidden_states, attention_mask, w_qkv, w_out)
    if KP not in _NC_CACHE:
        _NC_CACHE[KP] = build_kernel(KP)
    nc = _NC_CACHE[KP]
    res = bass_utils.run_bass_kernel_spmd(nc, in_maps,
                                          core_ids=list(range(NCORES)))
    out = np.empty((B, S, H), dtype=np.float32)
    for b in range(B):
        acc = res.results[b * CPB]["outT"].astype(np.float32).copy()
        for c in range(b * CPB + 1, (b + 1) * CPB):
            acc += res.results[c]["outT"]
        out[b] = acc.T
    return out


# revision 41
# speedup vs baseline: 1.0070x; 1.0070x over previous
"""Bass/Trainium2 kernel for nn_BaseAttention (B=2, S=2048, H=1024, NH=16, HD=64).

Sharding: 8 cores = 2 batches x 4 head-groups (4 heads each core).
Each core computes, for its (batch b, head-group hb):
    qkv slice -> attention over packed masked keys -> partial out-projection
and writes partial^T [H, S].  Host sums the 4 partials per batch and
transposes.

v4 design (all-fp16 data path; cost-model-guided):
  * fp16 everywhere instead of bf16: same 1 cyc/row matmul throughput and
    identical DMA bytes, but 8x less quantization error -- the error budget
    is then dominated by the Schraudolph exp tiles alone (~1.1e-2).
  * AV computed in [q, d] orientation: stationary = exp'd score tile
    [128 keys, 128 q], moving = per-(kt, head) V tile [128 keys, 64].
    Matmul time is out_free x 1 cyc, so AV drops ~2x vs the [d^T, q]
    orientation; softmax denominators come from parallel 1-column matmuls
    against a ones-column into a shared PSUM bank.  PSUM start=True zeroes
    the whole 2KB bank, so interleaved sub-bank accumulation groups carry
    start only on the bank's first matmul and rely on pending-zero for the
    rest (skip_group_check).
  * Normalization is a reciprocal of the denominator row plus a broadcast
    tensor_tensor multiply (per-partition q); the [q, d] -> [d, q]
    transpose runs on the DMA XBAR (dma_start_transpose) for the first
    head-pair (latency-tolerant) and on the PE array (identity matmul into
    a 1-bank f16 PSUM tile, tiny Act/DVE evacs) for the last pair, which
    gates the out-projection.
  * Scores in S^T layout [key_part, q_free], one [128,512] PSUM bank per
    half so the shared "pss" tag rotates 5 deep; exp halves split across
    Act (accurate exp) and DVE (Schraudolph: uint16(round(s*A + B)) bits
    ARE fp16(exp(s)); saturates to +0 for masked keys) at a 20:16 ratio.
    GPSIMD cannot access PSUM, so Act+DVE carry the whole exp wall.
  * Phase schedule: warmup matmuls hold the PE p-state ramp until the DMA
    stream lands; K-projection chases the per-ht (wk, xpT) stream with two
    V key-tiles riding along; Q half 0; attention q[0:1024]; Q half 1 +
    out-projection columns 0:1024 (output DMA overlaps the second
    attention half); attention q[1024:2048] with held-back out-projection
    rows as PE filler around the final pair's normalize/transpose; out-
    projection columns 1024:2048 streams straight to the output DMA.
  * Masked keys packed on host (KP = ceil(max_count/128)*128); 1/sqrt(HD)
    folded into wq on the host; key-padding bias fused into exp.
"""

import numpy as np

import concourse.bass as bass
import concourse.mybir as mybir
import concourse.tile as tile
from concourse import bacc
from concourse import bass_utils

B, S, H = 2, 2048, 1024
NH, HD = 16, 64
SCALE = HD ** -0.5
NCORES = 8
CPB = NCORES // B          # cores per batch = 4
NHL = NH // CPB            # local heads per core = 4
QD = NHL * HD              # local head-dim total = 256
HT = H // 128              # k-tiles over hidden dim = 8
MT = QD // 128             # partition-tiles over local head dims = 2
PO = 2                     # query halves (1024 each)
QT = 8                     # 128-query tiles per half

F32 = mybir.dt.float32
F16 = mybir.dt.float16
U16 = mybir.dt.uint16
NPF16 = np.float16

# Schraudolph exp -> fp16 bits: u16 = round(s * AEXP + BEXP), saturating at
# 0.  AEXP = 2^10/ln2 (fp16 exponent LSB is bit 10); the -60 fraction-bias
# offset minimizes the max relative error over the sawtooth.
AEXP = 1024.0 / float(np.log(2.0))
BEXP = 15.0 * 1024.0 - 60.0


def _chunks(total, size):
    out = []
    o = 0
    while o < total:
        c = min(size, total - o)
        out.append((o, c))
        o += c
    return out


# Engines for the two 512-halves of one (head, kt) exp tile.
# 'a' = Act accurate exp, 'd' = DVE Schraudolph.  GPSIMD cannot access
# PSUM on TRN2, so the exp wall is carried by Act+DVE alone; the cycle
# of 9 gives Act 20 / DVE 16 halves per 9-kt pair loop.
_EXP_TABLE = [("a", "d"), ("d", "a"), ("a", "d"),
              ("d", "a"), ("a", "d"), ("d", "a"),
              ("a", "d"), ("d", "a"), ("a", "a")]


def _exp_engines(kt, hi, last=False):
    if last:
        return ("a", "d") if hi == 0 else ("d", "a")
    return _EXP_TABLE[(kt * 2 + hi) % 9]


def build_kernel(KP):
    KT = KP // 128
    nc = bacc.Bacc("TRN2")
    ident = nc.dram_tensor("ident", [128, 128], F16, kind="ExternalInput")
    xT = nc.dram_tensor("xT", [H, S], F16, kind="ExternalInput")
    xpT = nc.dram_tensor("xpT", [H, KP], F16, kind="ExternalInput")
    wqT = nc.dram_tensor("wqT", [H, QD], F16, kind="ExternalInput")
    wkT = nc.dram_tensor("wkT", [H, QD], F16, kind="ExternalInput")
    wvT = nc.dram_tensor("wvT", [H, QD], F16, kind="ExternalInput")
    woT = nc.dram_tensor("woT", [QD, H], F16, kind="ExternalInput")
    bk = nc.dram_tensor("bk", [128, KT], F32, kind="ExternalInput")
    bk2 = nc.dram_tensor("bk2", [128, KT], F32, kind="ExternalInput")
    outT = nc.dram_tensor("outT", [H, S], F16, kind="ExternalOutput")

    with tile.TileContext(nc) as tc:
        with tile.TileContext.tile_pool(tc, name="wts", bufs=1) as wp:
            wq_sb = wp.tile([128, HT, QD], F16)
            wk_sb = wp.tile([128, HT, QD], F16)
            wv_sb = wp.tile([128, HT, QD], F16)
            wo_sb = wp.tile([128, MT, H], F16)
            bk_sb = wp.tile([128, KT], F32)
            bk2_sb = wp.tile([128, KT], F32)
            xT_sb = wp.tile([128, HT, S], F16)
            xpT_sb = wp.tile([128, HT, KP], F16)
            qT_sb = wp.tile([128, MT, S], F16)
            kT_sb = wp.tile([128, MT, KP], F16)
            va_sb = wp.tile([128, KT, NHL, 65], F16)   # V rows + ones col
            aT_sb = wp.tile([128, MT, S], F16)
            id_sb = wp.tile([128, 128], F16)

            wu_sb = wp.tile([128, 128], F16)
            nc.vector.memset(wu_sb, 0.0)

            # --- input DMA on two queues: wk slices + small tensors on the
            # scalar queue, the bulk stream (xpT, wv, wq, xT, wo) on sync.
            # The two queues land wk[ht0] and xpT[ht0] in parallel so the
            # K projection's first matmul starts ~3.7us in and chases the
            # per-ht stream.
            nc.sync.dma_start(out=wk_sb,
                              in_=wkT.ap().rearrange("(t p) d -> p t d",
                                                     p=128))
            for ht in range(HT):
                nc.sync.dma_start(out=xpT_sb[:, ht, :],
                                  in_=xpT.ap()[ht * 128:(ht + 1) * 128, :])
                if ht == 5:
                    nc.sync.dma_start(
                        out=wv_sb,
                        in_=wvT.ap().rearrange("(t p) d -> p t d", p=128))
            nc.scalar.dma_start(out=bk_sb, in_=bk.ap())
            nc.scalar.dma_start(out=bk2_sb, in_=bk2.ap())
            nc.scalar.dma_start(out=id_sb, in_=ident.ap())
            nc.sync.dma_start(out=wq_sb,
                              in_=wqT.ap().rearrange("(t p) d -> p t d",
                                                     p=128))
            for hp in range(HT // 2):
                nc.sync.dma_start(
                    out=xT_sb[:, 2 * hp:2 * hp + 2, :],
                    in_=xT.ap()[hp * 256:(hp + 1) * 256, :].rearrange(
                        "(t p) s -> p t s", p=128))
            nc.sync.dma_start(out=wo_sb,
                              in_=woT.ap().rearrange("(t p) d -> p t d",
                                                     p=128))
            nc.vector.memset(va_sb[:, :, :, 64:65], 1.0)

            evac_flip = [0]

            def evac(dst, src):
                # alternate psum evacuations between Act and DVE
                if evac_flip[0] % 2 == 0:
                    nc.scalar.copy(dst, src)
                else:
                    nc.vector.tensor_copy(dst, src)
                evac_flip[0] += 1

            # single PSUM pool, 8 banks: tag "pss" = 5 rotating [128,512]f32
            # banks (projections / scores / out-proj / PE-transpose spill),
            # tag "pav" = 2 banks (AV accumulators, K remainder chunks, V
            # projection), tag "pdn" = 1 bank (softmax denominators).
            with tile.TileContext.tile_pool(tc, name="pss", bufs=5,
                                            space="PSUM") as pss:
                def pstile(shape, tag, bufs, name, dt=F32):
                    return pss.tile(shape, dt, tag=tag, bufs=bufs, name=name)

                if True:
                    # ---- PE warmup: tiny matmuls on zeros keep the tensor
                    # engine's p-state ramp running until the first real
                    # matmul's inputs land (~4.5us), so the K projection
                    # starts at full clock.
                    wps = pstile([128, 512], "pss", 5, "ps_wu")
                    for _ in range(40):
                        nc.tensor.matmul(wps[:, 0:128], wu_sb, wu_sb,
                                         start=True, stop=True)

                    # ---- K^T projection, ht-outer so matmuls chase the DMA.
                    kchunks = []
                    for mt in range(MT):
                        for po, pw in _chunks(KP, 512):
                            if pw > 128:
                                ps = pstile([128, 512], "pss", 5,
                                            f"ps_k{mt}_{po}")
                            else:
                                ps = pstile([128, 128], "pav", 2,
                                            f"ps_k{mt}_{po}")
                            kchunks.append((mt, po, pw, ps))
                    # the first two V-projection key-tiles ride along in
                    # the K ht-loop so the PE outpaces the xpT DMA stream
                    vps = [pstile([128, QD], "pss", 5, "ps_v0"),
                           pstile([128, QD], "pdn", 1, "ps_v1")]
                    for ht in range(HT):
                        for mt, po, pw, ps in kchunks:
                            nc.tensor.matmul(
                                ps[:, 0:pw],
                                wk_sb[:, ht, mt * 128:(mt + 1) * 128],
                                xpT_sb[:, ht, po:po + pw],
                                start=(ht == 0), stop=(ht == HT - 1))
                        for st in range(2):
                            nc.tensor.matmul(
                                vps[st],
                                xpT_sb[:, ht, st * 128:(st + 1) * 128],
                                wv_sb[:, ht, :],
                                start=(ht == 0), stop=(ht == HT - 1))
                    for mt, po, pw, ps in sorted(kchunks,
                                                 key=lambda c: -c[1]):
                        evac(kT_sb[:, mt, po:po + pw], ps[:, 0:pw])
                    for st in range(2):
                        evac(va_sb[:, st, :, 0:64],
                             vps[st].rearrange("p (h d) -> p h d", h=NHL))

                    # ---- V projection, remaining key-tiles
                    for st in range(2, KT):
                        pv = pstile([128, QD], "pav", 2, "ps_v")
                        for ht in range(HT):
                            nc.tensor.matmul(
                                pv, xpT_sb[:, ht, st * 128:(st + 1) * 128],
                                wv_sb[:, ht, :],
                                start=(ht == 0), stop=(ht == HT - 1))
                        evac(va_sb[:, st, :, 0:64],
                             pv.rearrange("p (h d) -> p h d", h=NHL))

                # ---- Q^T projection for a query half
                def emit_q(po, mts=(0, 1)):
                    for mt in mts:
                        for co, cw in _chunks(1024, 512):
                            ps = pstile([128, 512], "pss", 5,
                                        f"ps_q{mt}_{po + co}")
                            for ht in range(HT):
                                nc.tensor.matmul(
                                    ps,
                                    wq_sb[:, ht, mt * 128:(mt + 1) * 128],
                                    xT_sb[:, ht, po + co:po + co + cw],
                                    start=(ht == 0), stop=(ht == HT - 1))
                            evac(qT_sb[:, mt, po + co:po + co + cw], ps)

                emit_q(0)

                # ---- attention + interleaved out-projection phases
                with tile.TileContext.tile_pool(tc, name="pex", bufs=12) as pxp, \
                     tile.TileContext.tile_pool(tc, name="an", bufs=3) as anp, \
                     tile.TileContext.tile_pool(tc, name="rc", bufs=4) as rcp, \
                     tile.TileContext.tile_pool(tc, name="stg", bufs=10) as sgp:

                    def attn_pair(pair, po):
                        """Attention for heads `pair` on queries
                        [po*1024, (po+1)*1024)."""
                        mtq = pair[0] // 2
                        q0 = po * 1024
                        pav = {}
                        pend = {h: [] for h in pair}
                        for h in pair:
                            pav[h] = pstile([128, QT, 64], "pav", 2,
                                            f"pav{h}_{po}")
                        # softmax denominators for both heads (64B bank)
                        pden = pstile([128, 2, QT], "pdn", 1,
                                      f"pdn{pair[0]}_{po}")

                        def flush_av(hi, h):
                            # start=True zeroes the whole 2KB PSUM bank, so
                            # only the very first matmul into each bank may
                            # carry it; the other interleaved accumulation
                            # groups land on pending-zero bytes (zeroed on
                            # first write).
                            pkt, ppx = pend[h].pop(0)
                            for qt in range(QT):
                                pxs = ppx[:, qt * 128:(qt + 1) * 128]
                                nc.tensor.matmul(
                                    pav[h][:, qt, :], pxs,
                                    va_sb[:, pkt, h, 0:64],
                                    start=(pkt == 0 and qt == 0),
                                    stop=(pkt == KT - 1 and qt == QT - 1),
                                    skip_group_check=True)
                                nc.tensor.matmul(
                                    pden[:, hi, qt:qt + 1], pxs,
                                    va_sb[:, pkt, h, 64:65],
                                    start=(pkt == 0 and qt == 0 and hi == 0),
                                    stop=(pkt == KT - 1 and qt == QT - 1
                                          and hi == 1),
                                    skip_group_check=True)

                        for kt in range(KT):
                            for hi, h in enumerate(pair):
                                rb = (h * HD) % 128
                                px = pxp.tile([128, 1024], F16, tag="pex",
                                              name=f"pex{h}_{po}_{kt}")
                                engs = _exp_engines(kt, hi, last=(kt == KT - 1))
                                for ci, (co, cw) in enumerate(_chunks(1024, 512)):
                                    ps = pstile([128, 512], "pss", 5,
                                                f"ps_s{h}_{po}_{kt}_{co}")
                                    nc.tensor.matmul(
                                        ps,
                                        kT_sb[rb:rb + HD, mtq,
                                              kt * 128:(kt + 1) * 128],
                                        qT_sb[rb:rb + HD, mtq,
                                              q0 + co:q0 + co + cw],
                                        start=True, stop=True)
                                    eng = engs[ci]
                                    if eng == "a":
                                        nc.scalar.activation(
                                            out=px[:, co:co + cw],
                                            in_=ps,
                                            func=mybir.ActivationFunctionType.Exp,
                                            bias=bk_sb[:, kt:kt + 1], scale=1.0)
                                    else:
                                        nc.vector.tensor_scalar(
                                            px[:, co:co + cw].bitcast(U16),
                                            ps,
                                            AEXP, bk2_sb[:, kt:kt + 1],
                                            mybir.AluOpType.mult,
                                            mybir.AluOpType.add)
                                pend[h].append((kt, px))
                            # drain the AV backlog harder near the end of the
                            # kt loop so normalize/transpose start promptly
                            max_pend = 2 if kt < KT - 2 else (KT - 1 - kt)
                            for hi, h in enumerate(pair):
                                while len(pend[h]) > max_pend:
                                    flush_av(hi, h)
                        an = anp.tile([128, QT, 128], F16, tag="an",
                                      name=f"an{pair[0]}_{po}")
                        rcb = {}
                        for hi, h in enumerate(pair):
                            while pend[h]:
                                flush_av(hi, h)
                            rc = rcp.tile([128, QT], F32, tag="rc",
                                          name=f"rc{h}_{po}")
                            nc.vector.reciprocal(rc, pden[:, hi, :])
                            r = rc.rearrange("p (q o) -> p q o", o=1)
                            rcb[h] = r.broadcast_to([128, QT, 64])
                        # normalize a_n[q, qt, d] = pav[q, qt, d] / den in
                        # qt-half groups so transposes can start early
                        for hi, h in enumerate(pair):
                            nc.vector.tensor_tensor(
                                out=an[:, :, hi * 64:hi * 64 + 64],
                                in0=pav[h][:, :, :], in1=rcb[h],
                                op=mybir.AluOpType.mult)

                        # transpose [q, d] -> [d, q].  The first pair rides
                        # the DMA XBAR (latency-tolerant: a full pair of
                        # compute follows); the last pair, which gates the
                        # out-projection, uses the PE array + tiny evacs.
                        # Emission is deferred to the returned closure so the
                        # caller can slot PE filler before the PE transposes.
                        tr_tile = []

                        def finish_half(hq):
                            qts = range(hq * (QT // 2), (hq + 1) * (QT // 2))
                            if mtq == 0:
                                for qt in qts:
                                    nc.sync.dma_start_transpose(
                                        aT_sb[:, mtq,
                                              q0 + qt * 128:q0 + (qt + 1) * 128],
                                        an[:, qt, :])
                                return
                            if not tr_tile:
                                tr_tile.append(pstile([128, QT, 128], "pav", 2,
                                                      f"tr{po}", dt=F16))
                            tr = tr_tile[0]
                            for qt in qts:
                                # sub-bank writes: only the first transpose
                                # into the tr bank may carry start=True
                                nc.tensor.matmul(
                                    tr[:, qt, :], an[:, qt, :], id_sb,
                                    is_transpose=True,
                                    start=(hq == 0 and qt == qts[0]),
                                    stop=(hq == 1 and qt == qts[-1]),
                                    skip_group_check=True)
                            engs = ((nc.scalar, nc.vector, nc.scalar,
                                     nc.vector) if hq == 0 else
                                    (nc.vector, nc.scalar, nc.vector,
                                     nc.scalar))
                            for qt in qts:
                                dst = aT_sb[:, mtq,
                                            q0 + qt * 128:q0 + (qt + 1) * 128]
                                copy_half(engs[qt % 4], dst, tr[:, qt, :])

                        def finish():
                            finish_half(0)
                            finish_half(1)

                        finish.half = finish_half
                        return finish

                    def copy_half(eng, dst, src):
                        if eng is nc.scalar:
                            eng.copy(dst, src)
                        else:
                            eng.tensor_copy(dst, src)

                    def out_proj(ho, jts, eoff=0):
                        """Out-projection rows jts, columns [ho*1024, +1024).
                        PSUM evacuation runs as 512-halves on two engines so
                        the pss slots free at PE pace."""
                        q0 = ho * 1024
                        for ji, jt in enumerate(jts):
                            stg = sgp.tile([128, 1024], F16, tag="stg",
                                           name="stage")
                            e01 = ((nc.scalar, nc.vector)
                                   if (ji + eoff) % 2 == 0
                                   else (nc.vector, nc.scalar))
                            for ci, (co, cw) in enumerate(_chunks(1024, 512)):
                                pf = pstile([128, 512], "pss", 5,
                                            f"ps_f{jt}_{ho}_{co}")
                                for mt in range(MT):
                                    nc.tensor.matmul(
                                        pf,
                                        wo_sb[:, mt, jt * 128:(jt + 1) * 128],
                                        aT_sb[:, mt, q0 + co:q0 + co + cw],
                                        start=(mt == 0), stop=(mt == MT - 1))
                                copy_half(e01[ci], stg[:, co:co + cw], pf)
                            nc.sync.dma_start(
                                out=outT.ap()[jt * 128:(jt + 1) * 128,
                                              q0:q0 + 1024],
                                in_=stg)

                    attn_pair((0, 1), 0)()
                    f2 = attn_pair((2, 3), 0)
                    emit_q(1024, mts=(0,))   # PE filler while normalize lands
                    f2()
                    emit_q(1024, mts=(1,))
                    out_proj(0, range(5))
                    attn_pair((0, 1), 1)()
                    f4 = attn_pair((2, 3), 1)
                    # held-back ho=0 rows fill the PE while the last pair's
                    # normalize lands, then its PE transposes + evacs run
                    out_proj(0, range(5, 6), eoff=1)
                    f4.half(0)
                    out_proj(0, range(6, HT), eoff=1)
                    f4.half(1)
                    out_proj(1, range(HT))

    nc.compile()
    return nc


def _prep_inputs(hidden_states, attention_mask, w_qkv, w_out):
    """Shard + transpose + quantize inputs for the 8 cores."""
    hs = np.asarray(hidden_states, dtype=np.float32)
    mask = np.asarray(attention_mask)
    wqkv = np.asarray(w_qkv, dtype=np.float32)
    wo = np.asarray(w_out, dtype=np.float32)

    idxs = [np.nonzero(mask[b] != 0)[0] for b in range(B)]
    counts = [len(ix) for ix in idxs]
    KP = max(128, ((max(counts) + 127) // 128) * 128)
    KT = KP // 128

    xTs, xpTs, bks, bk2s = [], [], [], []
    for b in range(B):
        xb = hs[b].astype(NPF16)
        xTs.append(np.ascontiguousarray(xb.T))
        xp = np.zeros((KP, H), dtype=NPF16)
        xp[:counts[b]] = xb[idxs[b]]
        xpTs.append(np.ascontiguousarray(xp.T))
        bias = np.zeros(KP, dtype=np.float32)
        bias[counts[b]:] = -30000.0
        bias = np.ascontiguousarray(bias.reshape(KT, 128).T)
        bks.append(bias)
        bk2s.append(np.ascontiguousarray(
            (bias * AEXP + BEXP).astype(np.float32)))

    ident = np.ascontiguousarray(np.eye(128, dtype=NPF16))
    in_maps = []
    for c in range(NCORES):
        b, hb = c // CPB, c % CPB
        sl = slice(hb * QD, (hb + 1) * QD)
        in_maps.append({
            "ident": ident,
            "xT": xTs[b],
            "xpT": xpTs[b],
            "wqT": np.ascontiguousarray(
                (wqkv[sl, :] * SCALE).astype(NPF16).T),
            "wkT": np.ascontiguousarray(
                wqkv[H + sl.start:H + sl.stop, :].astype(NPF16).T),
            "wvT": np.ascontiguousarray(
                wqkv[2 * H + sl.start:2 * H + sl.stop, :].astype(NPF16).T),
            "woT": np.ascontiguousarray(wo[:, sl].astype(NPF16).T),
            "bk": bks[b],
            "bk2": bk2s[b],
        })
    return KP, in_maps


_NC_CACHE = {}


def kernel(hidden_states, attention_mask, w_qkv, w_out):
    KP, in_maps = _prep_inputs(hidden_states, attention_mask, w_qkv, w_out)
    if KP not in _NC_CACHE:
        _NC_CACHE[KP] = build_kernel(KP)
    nc = _NC_CACHE[KP]
    res = bass_utils.run_bass_kernel_spmd(nc, in_maps,
                                          core_ids=list(range(NCORES)))
    out = np.empty((B, S, H), dtype=np.float32)
    for b in range(B):
        acc = res.results[b * CPB]["outT"].astype(np.float32).copy()
        for c in range(b * CPB + 1, (b + 1) * CPB):
            acc += res.results[c]["outT"]
        out[b] = acc.T
    return out


# revision 44
# speedup vs baseline: 1.0099x; 1.0029x over previous
"""Bass/Trainium2 kernel for nn_BaseAttention (B=2, S=2048, H=1024, NH=16, HD=64).

Sharding: 8 cores = 2 batches x 4 head-groups (4 heads each core).
Each core computes, for its (batch b, head-group hb):
    qkv slice -> attention over packed masked keys -> partial out-projection
and writes partial^T [H, S].  Host sums the 4 partials per batch and
transposes.

v4 design (all-fp16 data path; cost-model-guided):
  * fp16 everywhere instead of bf16: same 1 cyc/row matmul throughput and
    identical DMA bytes, but 8x less quantization error -- the error budget
    is then dominated by the Schraudolph exp tiles alone (~1.1e-2).
  * AV computed in [q, d] orientation: stationary = exp'd score tile
    [128 keys, 128 q], moving = per-(kt, head) V tile [128 keys, 64].
    Matmul time is out_free x 1 cyc, so AV drops ~2x vs the [d^T, q]
    orientation; softmax denominators come from parallel 1-column matmuls
    against a ones-column into a shared PSUM bank.  PSUM start=True zeroes
    the whole 2KB bank, so interleaved sub-bank accumulation groups carry
    start only on the bank's first matmul and rely on pending-zero for the
    rest (skip_group_check).
  * Normalization is a reciprocal of the denominator row plus a broadcast
    tensor_tensor multiply (per-partition q); the [q, d] -> [d, q]
    transpose runs on the DMA XBAR (dma_start_transpose) for the first
    head-pair (latency-tolerant) and on the PE array (identity matmul into
    a 1-bank f16 PSUM tile, tiny Act/DVE evacs) for the last pair, which
    gates the out-projection.
  * Scores in S^T layout [key_part, q_free], one [128,512] PSUM bank per
    half so the shared "pss" tag rotates 5 deep; exp halves split across
    Act (accurate exp) and DVE (Schraudolph: uint16(round(s*A + B)) bits
    ARE fp16(exp(s)); saturates to +0 for masked keys) at a 20:16 ratio.
    GPSIMD cannot access PSUM, so Act+DVE carry the whole exp wall.
  * Phase schedule: warmup matmuls hold the PE p-state ramp until the DMA
    stream lands; K-projection chases the per-ht (wk, xpT) stream with two
    V key-tiles riding along; Q half 0; attention q[0:1024]; Q half 1 +
    out-projection columns 0:1024 (output DMA overlaps the second
    attention half); attention q[1024:2048] with held-back out-projection
    rows as PE filler around the final pair's normalize/transpose; out-
    projection columns 1024:2048 streams straight to the output DMA.
  * Masked keys packed on host (KP = ceil(max_count/128)*128); 1/sqrt(HD)
    folded into wq on the host; key-padding bias fused into exp.
"""

import numpy as np

import concourse.bass as bass
import concourse.mybir as mybir
import concourse.tile as tile
from concourse import bacc
from concourse import bass_utils

B, S, H = 2, 2048, 1024
NH, HD = 16, 64
SCALE = HD ** -0.5
NCORES = 8
CPB = NCORES // B          # cores per batch = 4
NHL = NH // CPB            # local heads per core = 4
QD = NHL * HD              # local head-dim total = 256
HT = H // 128              # k-tiles over hidden dim = 8
MT = QD // 128             # partition-tiles over local head dims = 2
PO = 2                     # query halves (1024 each)
QT = 8                     # 128-query tiles per half

F32 = mybir.dt.float32
F16 = mybir.dt.float16
U16 = mybir.dt.uint16
NPF16 = np.float16

# Schraudolph exp -> fp16 bits: u16 = round(s * AEXP + BEXP), saturating at
# 0.  AEXP = 2^10/ln2 (fp16 exponent LSB is bit 10); the -60 fraction-bias
# offset minimizes the max relative error over the sawtooth.
AEXP = 1024.0 / float(np.log(2.0))
BEXP = 15.0 * 1024.0 - 60.0


def _chunks(total, size):
    out = []
    o = 0
    while o < total:
        c = min(size, total - o)
        out.append((o, c))
        o += c
    return out


# Engines for the two 512-halves of one (head, kt) exp tile.
# 'a' = Act accurate exp, 'd' = DVE Schraudolph.  GPSIMD cannot access
# PSUM on TRN2, so the exp wall is carried by Act+DVE alone; the cycle
# of 9 gives Act 20 / DVE 16 halves per 9-kt pair loop.
_EXP_TABLE = [("a", "d"), ("d", "a"), ("a", "d"),
              ("d", "a"), ("a", "d"), ("d", "a"),
              ("a", "d"), ("d", "a"), ("a", "a")]


def _exp_engines(kt, hi, last=False):
    if last:
        return ("a", "d") if hi == 0 else ("d", "a")
    return _EXP_TABLE[(kt * 2 + hi) % 9]


def build_kernel(KP):
    KT = KP // 128
    nc = bacc.Bacc("TRN2")
    ident = nc.dram_tensor("ident", [128, 128], F16, kind="ExternalInput")
    xT = nc.dram_tensor("xT", [H, S], F16, kind="ExternalInput")
    xpT = nc.dram_tensor("xpT", [H, KP], F16, kind="ExternalInput")
    wqT = nc.dram_tensor("wqT", [H, QD], F16, kind="ExternalInput")
    wkT = nc.dram_tensor("wkT", [H, QD], F16, kind="ExternalInput")
    wvT = nc.dram_tensor("wvT", [H, QD], F16, kind="ExternalInput")
    woT = nc.dram_tensor("woT", [QD, H], F16, kind="ExternalInput")
    bk = nc.dram_tensor("bk", [128, KT], F32, kind="ExternalInput")
    bk2 = nc.dram_tensor("bk2", [128, KT], F32, kind="ExternalInput")
    outT = nc.dram_tensor("outT", [H, S], F16, kind="ExternalOutput")

    with tile.TileContext(nc) as tc:
        with tile.TileContext.tile_pool(tc, name="wts", bufs=1) as wp:
            wq_sb = wp.tile([128, HT, QD], F16)
            wk_sb = wp.tile([128, HT, QD], F16)
            wv_sb = wp.tile([128, HT, QD], F16)
            wo_sb = wp.tile([128, MT, H], F16)
            bk_sb = wp.tile([128, KT], F32)
            bk2_sb = wp.tile([128, KT], F32)
            xT_sb = wp.tile([128, HT, S], F16)
            xpT_sb = wp.tile([128, HT, KP], F16)
            qT_sb = wp.tile([128, MT, S], F16)
            kT_sb = wp.tile([128, MT, KP], F16)
            va_sb = wp.tile([128, KT, NHL, 65], F16)   # V rows + ones col
            aT_sb = wp.tile([128, MT, S], F16)
            id_sb = wp.tile([128, 128], F16)

            wu_sb = wp.tile([128, 128], F16)
            nc.vector.memset(wu_sb, 0.0)

            # --- input DMA on two queues: wk slices + small tensors on the
            # scalar queue, the bulk stream (xpT, wv, wq, xT, wo) on sync.
            # The two queues land wk[ht0] and xpT[ht0] in parallel so the
            # K projection's first matmul starts ~3.7us in and chases the
            # per-ht stream.
            nc.sync.dma_start(out=wk_sb,
                              in_=wkT.ap().rearrange("(t p) d -> p t d",
                                                     p=128))
            for ht in range(HT):
                nc.sync.dma_start(out=xpT_sb[:, ht, :],
                                  in_=xpT.ap()[ht * 128:(ht + 1) * 128, :])
                if ht == 5:
                    nc.sync.dma_start(
                        out=wv_sb,
                        in_=wvT.ap().rearrange("(t p) d -> p t d", p=128))
            nc.scalar.dma_start(out=bk_sb, in_=bk.ap())
            nc.scalar.dma_start(out=bk2_sb, in_=bk2.ap())
            nc.scalar.dma_start(out=id_sb, in_=ident.ap())
            nc.sync.dma_start(out=wq_sb,
                              in_=wqT.ap().rearrange("(t p) d -> p t d",
                                                     p=128))
            for hp in range(HT // 2):
                nc.sync.dma_start(
                    out=xT_sb[:, 2 * hp:2 * hp + 2, :],
                    in_=xT.ap()[hp * 256:(hp + 1) * 256, :].rearrange(
                        "(t p) s -> p t s", p=128))
            nc.sync.dma_start(out=wo_sb,
                              in_=woT.ap().rearrange("(t p) d -> p t d",
                                                     p=128))
            nc.vector.memset(va_sb[:, :, :, 64:65], 1.0)

            evac_flip = [0]

            def evac(dst, src):
                # alternate psum evacuations between Act and DVE
                if evac_flip[0] % 2 == 0:
                    nc.scalar.copy(dst, src)
                else:
                    nc.vector.tensor_copy(dst, src)
                evac_flip[0] += 1

            # single PSUM pool, 8 banks: tag "pss" = 5 rotating [128,512]f32
            # banks (projections / scores / out-proj / PE-transpose spill),
            # tag "pav" = 2 banks (AV accumulators, K remainder chunks, V
            # projection), tag "pdn" = 1 bank (softmax denominators).
            with tile.TileContext.tile_pool(tc, name="pss", bufs=5,
                                            space="PSUM") as pss:
                def pstile(shape, tag, bufs, name, dt=F32):
                    return pss.tile(shape, dt, tag=tag, bufs=bufs, name=name)

                if True:
                    # ---- PE warmup: tiny matmuls on zeros keep the tensor
                    # engine's p-state ramp running until the first real
                    # matmul's inputs land (~4.5us), so the K projection
                    # starts at full clock.
                    wps = pstile([128, 512], "pss", 5, "ps_wu")
                    for _ in range(40):
                        nc.tensor.matmul(wps[:, 0:128], wu_sb, wu_sb,
                                         start=True, stop=True)

                    # ---- K^T projection, ht-outer so matmuls chase the DMA.
                    kchunks = []
                    for mt in range(MT):
                        for po, pw in _chunks(KP, 512):
                            if pw > 128:
                                ps = pstile([128, 512], "pss", 5,
                                            f"ps_k{mt}_{po}")
                            else:
                                ps = pstile([128, 128], "pav", 2,
                                            f"ps_k{mt}_{po}")
                            kchunks.append((mt, po, pw, ps))
                    # the first two V-projection key-tiles ride along in
                    # the K ht-loop so the PE outpaces the xpT DMA stream
                    vps = [pstile([128, QD], "pss", 5, "ps_v0"),
                           pstile([128, QD], "pdn", 1, "ps_v1")]
                    for ht in range(HT):
                        for mt, po, pw, ps in kchunks:
                            nc.tensor.matmul(
                                ps[:, 0:pw],
                                wk_sb[:, ht, mt * 128:(mt + 1) * 128],
                                xpT_sb[:, ht, po:po + pw],
                                start=(ht == 0), stop=(ht == HT - 1))
                        for st in range(2):
                            nc.tensor.matmul(
                                vps[st],
                                xpT_sb[:, ht, st * 128:(st + 1) * 128],
                                wv_sb[:, ht, :],
                                start=(ht == 0), stop=(ht == HT - 1))
                    for mt, po, pw, ps in sorted(kchunks,
                                                 key=lambda c: -c[1]):
                        evac(kT_sb[:, mt, po:po + pw], ps[:, 0:pw])
                    for st in range(2):
                        evac(va_sb[:, st, :, 0:64],
                             vps[st].rearrange("p (h d) -> p h d", h=NHL))

                    # ---- V projection, remaining key-tiles
                    for st in range(2, KT):
                        pv = pstile([128, QD], "pav", 2, "ps_v")
                        for ht in range(HT):
                            nc.tensor.matmul(
                                pv, xpT_sb[:, ht, st * 128:(st + 1) * 128],
                                wv_sb[:, ht, :],
                                start=(ht == 0), stop=(ht == HT - 1))
                        evac(va_sb[:, st, :, 0:64],
                             pv.rearrange("p (h d) -> p h d", h=NHL))

                # ---- Q^T projection for a query half
                def emit_q(po, mts=(0, 1)):
                    for mt in mts:
                        for co, cw in _chunks(1024, 512):
                            ps = pstile([128, 512], "pss", 5,
                                        f"ps_q{mt}_{po + co}")
                            for ht in range(HT):
                                nc.tensor.matmul(
                                    ps,
                                    wq_sb[:, ht, mt * 128:(mt + 1) * 128],
                                    xT_sb[:, ht, po + co:po + co + cw],
                                    start=(ht == 0), stop=(ht == HT - 1))
                            evac(qT_sb[:, mt, po + co:po + co + cw], ps)

                emit_q(0)

                # ---- attention + interleaved out-projection phases
                with tile.TileContext.tile_pool(tc, name="pex", bufs=12) as pxp, \
                     tile.TileContext.tile_pool(tc, name="an", bufs=3) as anp, \
                     tile.TileContext.tile_pool(tc, name="rc", bufs=4) as rcp, \
                     tile.TileContext.tile_pool(tc, name="stg", bufs=10) as sgp:

                    def attn_pair(pair, po):
                        """Attention for heads `pair` on queries
                        [po*1024, (po+1)*1024)."""
                        mtq = pair[0] // 2
                        q0 = po * 1024
                        pav = {}
                        pend = {h: [] for h in pair}
                        for h in pair:
                            pav[h] = pstile([128, QT, 64], "pav", 2,
                                            f"pav{h}_{po}")
                        # softmax denominators for both heads (64B bank)
                        pden = pstile([128, 2, QT], "pdn", 1,
                                      f"pdn{pair[0]}_{po}")

                        def flush_av(hi, h):
                            # start=True zeroes the whole 2KB PSUM bank, so
                            # only the very first matmul into each bank may
                            # carry it; the other interleaved accumulation
                            # groups land on pending-zero bytes (zeroed on
                            # first write).
                            pkt, ppx = pend[h].pop(0)
                            for qt in range(QT):
                                pxs = ppx[:, qt * 128:(qt + 1) * 128]
                                nc.tensor.matmul(
                                    pav[h][:, qt, :], pxs,
                                    va_sb[:, pkt, h, 0:64],
                                    start=(pkt == 0 and qt == 0),
                                    stop=(pkt == KT - 1 and qt == QT - 1),
                                    skip_group_check=True)
                                nc.tensor.matmul(
                                    pden[:, hi, qt:qt + 1], pxs,
                                    va_sb[:, pkt, h, 64:65],
                                    start=(pkt == 0 and qt == 0 and hi == 0),
                                    stop=(pkt == KT - 1 and qt == QT - 1
                                          and hi == 1),
                                    skip_group_check=True)

                        for kt in range(KT):
                            pxs = {}
                            for hi, h in enumerate(pair):
                                pxs[h] = pxp.tile([128, 1024], F16, tag="pex",
                                                  name=f"pex{h}_{po}_{kt}")
                                pend[h].append((kt, pxs[h]))
                            for ci, (co, cw) in enumerate(_chunks(1024, 512)):
                                for hi, h in enumerate(pair):
                                    rb = (h * HD) % 128
                                    px = pxs[h]
                                    engs = _exp_engines(kt, hi,
                                                        last=(kt == KT - 1))
                                    ps = pstile([128, 512], "pss", 5,
                                                f"ps_s{h}_{po}_{kt}_{co}")
                                    nc.tensor.matmul(
                                        ps,
                                        kT_sb[rb:rb + HD, mtq,
                                              kt * 128:(kt + 1) * 128],
                                        qT_sb[rb:rb + HD, mtq,
                                              q0 + co:q0 + co + cw],
                                        start=True, stop=True)
                                    eng = engs[ci]
                                    if eng == "a":
                                        nc.scalar.activation(
                                            out=px[:, co:co + cw],
                                            in_=ps,
                                            func=mybir.ActivationFunctionType.Exp,
                                            bias=bk_sb[:, kt:kt + 1], scale=1.0)
                                    else:
                                        nc.vector.tensor_scalar(
                                            px[:, co:co + cw].bitcast(U16),
                                            ps,
                                            AEXP, bk2_sb[:, kt:kt + 1],
                                            mybir.AluOpType.mult,
                                            mybir.AluOpType.add)
                            # drain the AV backlog harder near the end of the
                            # kt loop so normalize/transpose start promptly
                            max_pend = 2 if kt < KT - 2 else (KT - 1 - kt)
                            for hi, h in enumerate(pair):
                                while len(pend[h]) > max_pend:
                                    flush_av(hi, h)
                        an = anp.tile([128, QT, 128], F16, tag="an",
                                      name=f"an{pair[0]}_{po}")
                        rcb = {}
                        for hi, h in enumerate(pair):
                            while pend[h]:
                                flush_av(hi, h)
                            rc = rcp.tile([128, QT], F32, tag="rc",
                                          name=f"rc{h}_{po}")
                            nc.vector.reciprocal(rc, pden[:, hi, :])
                            r = rc.rearrange("p (q o) -> p q o", o=1)
                            rcb[h] = r.broadcast_to([128, QT, 64])
                        # normalize a_n[q, qt, d] = pav[q, qt, d] / den in
                        # qt-half groups so transposes can start early
                        for hi, h in enumerate(pair):
                            nc.vector.tensor_tensor(
                                out=an[:, :, hi * 64:hi * 64 + 64],
                                in0=pav[h][:, :, :], in1=rcb[h],
                                op=mybir.AluOpType.mult)

                        # transpose [q, d] -> [d, q].  The first pair rides
                        # the DMA XBAR (latency-tolerant: a full pair of
                        # compute follows); the last pair, which gates the
                        # out-projection, uses the PE array + tiny evacs.
                        # Emission is deferred to the returned closure so the
                        # caller can slot PE filler before the PE transposes.
                        tr_tile = []

                        def finish_half(hq):
                            qts = range(hq * (QT // 2), (hq + 1) * (QT // 2))
                            if mtq == 0:
                                for qt in qts:
                                    nc.sync.dma_start_transpose(
                                        aT_sb[:, mtq,
                                              q0 + qt * 128:q0 + (qt + 1) * 128],
                                        an[:, qt, :])
                                return
                            if not tr_tile:
                                tr_tile.append(pstile([128, QT, 128], "pav", 2,
                                                      f"tr{po}", dt=F16))
                            tr = tr_tile[0]
                            for qt in qts:
                                # sub-bank writes: only the first transpose
                                # into the tr bank may carry start=True
                                nc.tensor.matmul(
                                    tr[:, qt, :], an[:, qt, :], id_sb,
                                    is_transpose=True,
                                    start=(hq == 0 and qt == qts[0]),
                                    stop=(hq == 1 and qt == qts[-1]),
                                    skip_group_check=True)
                            engs = ((nc.scalar, nc.vector, nc.scalar,
                                     nc.vector) if hq == 0 else
                                    (nc.vector, nc.scalar, nc.vector,
                                     nc.scalar))
                            for qt in qts:
                                dst = aT_sb[:, mtq,
                                            q0 + qt * 128:q0 + (qt + 1) * 128]
                                copy_half(engs[qt % 4], dst, tr[:, qt, :])

                        def finish():
                            finish_half(0)
                            finish_half(1)

                        finish.half = finish_half
                        return finish

                    def copy_half(eng, dst, src):
                        if eng is nc.scalar:
                            eng.copy(dst, src)
                        else:
                            eng.tensor_copy(dst, src)

                    def out_proj(ho, jts, eoff=0):
                        """Out-projection rows jts, columns [ho*1024, +1024).
                        PSUM evacuation runs as 512-halves on two engines so
                        the pss slots free at PE pace."""
                        q0 = ho * 1024
                        for ji, jt in enumerate(jts):
                            stg = sgp.tile([128, 1024], F16, tag="stg",
                                           name="stage")
                            e01 = ((nc.scalar, nc.vector)
                                   if (ji + eoff) % 2 == 0
                                   else (nc.vector, nc.scalar))
                            for ci, (co, cw) in enumerate(_chunks(1024, 512)):
                                pf = pstile([128, 512], "pss", 5,
                                            f"ps_f{jt}_{ho}_{co}")
                                for mt in range(MT):
                                    nc.tensor.matmul(
                                        pf,
                                        wo_sb[:, mt, jt * 128:(jt + 1) * 128],
                                        aT_sb[:, mt, q0 + co:q0 + co + cw],
                                        start=(mt == 0), stop=(mt == MT - 1))
                                copy_half(e01[ci], stg[:, co:co + cw], pf)
                            nc.sync.dma_start(
                                out=outT.ap()[jt * 128:(jt + 1) * 128,
                                              q0:q0 + 1024],
                                in_=stg)

                    attn_pair((0, 1), 0)()
                    f2 = attn_pair((2, 3), 0)
                    emit_q(1024, mts=(0,))   # PE filler while normalize lands
                    f2()
                    emit_q(1024, mts=(1,))
                    out_proj(0, range(5))
                    attn_pair((0, 1), 1)()
                    f4 = attn_pair((2, 3), 1)
                    # held-back ho=0 rows fill the PE while the last pair's
                    # normalize lands, then its PE transposes + evacs run
                    out_proj(0, range(5, 6), eoff=1)
                    f4.half(0)
                    out_proj(0, range(6, HT), eoff=1)
                    f4.half(1)
                    out_proj(1, range(HT))

    nc.compile()
    return nc


def _prep_inputs(hidden_states, attention_mask, w_qkv, w_out):
    """Shard + transpose + quantize inputs for the 8 cores."""
    hs = np.asarray(hidden_states, dtype=np.float32)
    mask = np.asarray(attention_mask)
    wqkv = np.asarray(w_qkv, dtype=np.float32)
    wo = np.asarray(w_out, dtype=np.float32)

    idxs = [np.nonzero(mask[b] != 0)[0] for b in range(B)]
    counts = [len(ix) for ix in idxs]
    KP = max(128, ((max(counts) + 127) // 128) * 128)
    KT = KP // 128

    xTs, xpTs, bks, bk2s = [], [], [], []
    for b in range(B):
        xb = hs[b].astype(NPF16)
        xTs.append(np.ascontiguousarray(xb.T))
        xp = np.zeros((KP, H), dtype=NPF16)
        xp[:counts[b]] = xb[idxs[b]]
        xpTs.append(np.ascontiguousarray(xp.T))
        bias = np.zeros(KP, dtype=np.float32)
        bias[counts[b]:] = -30000.0
        bias = np.ascontiguousarray(bias.reshape(KT, 128).T)
        bks.append(bias)
        bk2s.append(np.ascontiguousarray(
            (bias * AEXP + BEXP).astype(np.float32)))

    ident = np.ascontiguousarray(np.eye(128, dtype=NPF16))
    in_maps = []
    for c in range(NCORES):
        b, hb = c // CPB, c % CPB
        sl = slice(hb * QD, (hb + 1) * QD)
        in_maps.append({
            "ident": ident,
            "xT": xTs[b],
            "xpT": xpTs[b],
            "wqT": np.ascontiguousarray(
                (wqkv[sl, :] * SCALE).astype(NPF16).T),
            "wkT": np.ascontiguousarray(
                wqkv[H + sl.start:H + sl.stop, :].astype(NPF16).T),
            "wvT": np.ascontiguousarray(
                wqkv[2 * H + sl.start:2 * H + sl.stop, :].astype(NPF16).T),
            "woT": np.ascontiguousarray(wo[:, sl].astype(NPF16).T),
            "bk": bks[b],
            "bk2": bk2s[b],
        })
    return KP, in_maps


_NC_CACHE = {}


def kernel(hidden_states, attention_mask, w_qkv, w_out):
    KP, in_maps = _prep_inputs(hidden_states, attention_mask, w_qkv, w_out)
    if KP not in _NC_CACHE:
        _NC_CACHE[KP] = build_kernel(KP)
    nc = _NC_CACHE[KP]
    res = bass_utils.run_bass_kernel_spmd(nc, in_maps,
                                          core_ids=list(range(NCORES)))
    out = np.empty((B, S, H), dtype=np.float32)
    for b in range(B):
        acc = res.results[b * CPB]["outT"].astype(np.float32).copy()
        for c in range(b * CPB + 1, (b + 1) * CPB):
            acc += res.results[c]["outT"]
        out[b] = acc.T
    return out


# revision 46
# speedup vs baseline: 1.0126x; 1.0027x over previous
"""Bass/Trainium2 kernel for nn_BaseAttention (B=2, S=2048, H=1024, NH=16, HD=64).

Sharding: 8 cores = 2 batches x 4 head-groups (4 heads each core).
Each core computes, for its (batch b, head-group hb):
    qkv slice -> attention over packed masked keys -> partial out-projection
and writes partial^T [H, S].  Host sums the 4 partials per batch and
transposes.

v4 design (all-fp16 data path; cost-model-guided):
  * fp16 everywhere instead of bf16: same 1 cyc/row matmul throughput and
    identical DMA bytes, but 8x less quantization error -- the error budget
    is then dominated by the Schraudolph exp tiles alone (~1.1e-2).
  * AV computed in [q, d] orientation: stationary = exp'd score tile
    [128 keys, 128 q], moving = per-(kt, head) V tile [128 keys, 64].
    Matmul time is out_free x 1 cyc, so AV drops ~2x vs the [d^T, q]
    orientation; softmax denominators come from parallel 1-column matmuls
    against a ones-column into a shared PSUM bank.  PSUM start=True zeroes
    the whole 2KB bank, so interleaved sub-bank accumulation groups carry
    start only on the bank's first matmul and rely on pending-zero for the
    rest (skip_group_check).
  * Normalization is a reciprocal of the denominator row plus a broadcast
    tensor_tensor multiply (per-partition q); the [q, d] -> [d, q]
    transpose runs on the DMA XBAR (dma_start_transpose) for the first
    head-pair (latency-tolerant) and on the PE array (identity matmul into
    a 1-bank f16 PSUM tile, tiny Act/DVE evacs) for the last pair, which
    gates the out-projection.
  * Scores in S^T layout [key_part, q_free], one [128,512] PSUM bank per
    half so the shared "pss" tag rotates 5 deep; exp halves split across
    Act (accurate exp) and DVE (Schraudolph: uint16(round(s*A + B)) bits
    ARE fp16(exp(s)); saturates to +0 for masked keys) at a 20:16 ratio.
    GPSIMD cannot access PSUM, so Act+DVE carry the whole exp wall.
  * Phase schedule: warmup matmuls hold the PE p-state ramp until the DMA
    stream lands; K-projection chases the per-ht (wk, xpT) stream with two
    V key-tiles riding along; Q half 0; attention q[0:1024]; Q half 1 +
    out-projection columns 0:1024 (output DMA overlaps the second
    attention half); attention q[1024:2048] with held-back out-projection
    rows as PE filler around the final pair's normalize/transpose; out-
    projection columns 1024:2048 streams straight to the output DMA.
  * Masked keys packed on host (KP = ceil(max_count/128)*128); 1/sqrt(HD)
    folded into wq on the host; key-padding bias fused into exp.
"""

import numpy as np

import concourse.bass as bass
import concourse.mybir as mybir
import concourse.tile as tile
from concourse import bacc
from concourse import bass_utils

B, S, H = 2, 2048, 1024
NH, HD = 16, 64
SCALE = HD ** -0.5
NCORES = 8
CPB = NCORES // B          # cores per batch = 4
NHL = NH // CPB            # local heads per core = 4
QD = NHL * HD              # local head-dim total = 256
HT = H // 128              # k-tiles over hidden dim = 8
MT = QD // 128             # partition-tiles over local head dims = 2
PO = 2                     # query halves (1024 each)
QT = 8                     # 128-query tiles per half

F32 = mybir.dt.float32
F16 = mybir.dt.float16
U16 = mybir.dt.uint16
NPF16 = np.float16

# Schraudolph exp -> fp16 bits: u16 = round(s * AEXP + BEXP), saturating at
# 0.  AEXP = 2^10/ln2 (fp16 exponent LSB is bit 10); the -60 fraction-bias
# offset minimizes the max relative error over the sawtooth.
AEXP = 1024.0 / float(np.log(2.0))
BEXP = 15.0 * 1024.0 - 60.0


def _chunks(total, size):
    out = []
    o = 0
    while o < total:
        c = min(size, total - o)
        out.append((o, c))
        o += c
    return out


# Engines for the two 512-halves of one (head, kt) exp tile.
# 'a' = Act accurate exp, 'd' = DVE Schraudolph.  GPSIMD cannot access
# PSUM on TRN2, so the exp wall is carried by Act+DVE alone; the cycle
# of 9 gives Act 20 / DVE 16 halves per 9-kt pair loop.
_EXP_TABLE = [("a", "d"), ("d", "a"), ("a", "d"),
              ("d", "a"), ("a", "d"), ("d", "a"),
              ("a", "d"), ("d", "a"), ("a", "a")]


def _exp_engines(kt, hi, last=False):
    if last:
        return ("a", "d") if hi == 0 else ("d", "a")
    return _EXP_TABLE[(kt * 2 + hi) % 9]


def build_kernel(KP):
    KT = KP // 128
    nc = bacc.Bacc("TRN2")
    ident = nc.dram_tensor("ident", [128, 128], F16, kind="ExternalInput")
    xT = nc.dram_tensor("xT", [H, S], F16, kind="ExternalInput")
    xpT = nc.dram_tensor("xpT", [H, KP], F16, kind="ExternalInput")
    wqT = nc.dram_tensor("wqT", [H, QD], F16, kind="ExternalInput")
    wkT = nc.dram_tensor("wkT", [H, QD], F16, kind="ExternalInput")
    wvT = nc.dram_tensor("wvT", [H, QD], F16, kind="ExternalInput")
    woT = nc.dram_tensor("woT", [QD, H], F16, kind="ExternalInput")
    bk = nc.dram_tensor("bk", [128, KT], F32, kind="ExternalInput")
    bk2 = nc.dram_tensor("bk2", [128, KT], F32, kind="ExternalInput")
    outT = nc.dram_tensor("outT", [H, S], F16, kind="ExternalOutput")

    with tile.TileContext(nc) as tc:
        with tile.TileContext.tile_pool(tc, name="wts", bufs=1) as wp:
            wq_sb = wp.tile([128, HT, QD], F16)
            wk_sb = wp.tile([128, HT, QD], F16)
            wv_sb = wp.tile([128, HT, QD], F16)
            wo_sb = wp.tile([128, MT, H], F16)
            bk_sb = wp.tile([128, KT], F32)
            bk2_sb = wp.tile([128, KT], F32)
            xT_sb = wp.tile([128, HT, S], F16)
            xpT_sb = wp.tile([128, HT, KP], F16)
            qT_sb = wp.tile([128, MT, S], F16)
            kT_sb = wp.tile([128, MT, KP], F16)
            va_sb = wp.tile([128, KT, NHL, 65], F16)   # V rows + ones col
            aT_sb = wp.tile([128, MT, S], F16)
            id_sb = wp.tile([128, 128], F16)

            wu_sb = wp.tile([128, 128], F16)
            nc.vector.memset(wu_sb, 0.0)

            # --- input DMA on two queues: wk slices + small tensors on the
            # scalar queue, the bulk stream (xpT, wv, wq, xT, wo) on sync.
            # The two queues land wk[ht0] and xpT[ht0] in parallel so the
            # K projection's first matmul starts ~3.7us in and chases the
            # per-ht stream.
            nc.sync.dma_start(out=wk_sb,
                              in_=wkT.ap().rearrange("(t p) d -> p t d",
                                                     p=128))
            for ht in range(HT):
                nc.sync.dma_start(out=xpT_sb[:, ht, :],
                                  in_=xpT.ap()[ht * 128:(ht + 1) * 128, :])
                if ht == 5:
                    nc.sync.dma_start(
                        out=wv_sb,
                        in_=wvT.ap().rearrange("(t p) d -> p t d", p=128))
            nc.scalar.dma_start(out=bk_sb, in_=bk.ap())
            nc.scalar.dma_start(out=bk2_sb, in_=bk2.ap())
            nc.scalar.dma_start(out=id_sb, in_=ident.ap())
            nc.sync.dma_start(out=wq_sb,
                              in_=wqT.ap().rearrange("(t p) d -> p t d",
                                                     p=128))
            for hp in range(HT // 2):
                nc.sync.dma_start(
                    out=xT_sb[:, 2 * hp:2 * hp + 2, :],
                    in_=xT.ap()[hp * 256:(hp + 1) * 256, :].rearrange(
                        "(t p) s -> p t s", p=128))
            nc.sync.dma_start(out=wo_sb,
                              in_=woT.ap().rearrange("(t p) d -> p t d",
                                                     p=128))
            nc.vector.memset(va_sb[:, :, :, 64:65], 1.0)

            evac_flip = [0]

            def evac(dst, src):
                # alternate psum evacuations between Act and DVE
                if evac_flip[0] % 2 == 0:
                    nc.scalar.copy(dst, src)
                else:
                    nc.vector.tensor_copy(dst, src)
                evac_flip[0] += 1

            # single PSUM pool, 8 banks: tag "pss" = 5 rotating [128,512]f32
            # banks (projections / scores / out-proj / PE-transpose spill),
            # tag "pav" = 2 banks (AV accumulators, K remainder chunks, V
            # projection), tag "pdn" = 1 bank (softmax denominators).
            with tile.TileContext.tile_pool(tc, name="pss", bufs=5,
                                            space="PSUM") as pss:
                def pstile(shape, tag, bufs, name, dt=F32):
                    return pss.tile(shape, dt, tag=tag, bufs=bufs, name=name)

                if True:
                    # ---- PE warmup: tiny matmuls on zeros keep the tensor
                    # engine's p-state ramp running until the first real
                    # matmul's inputs land (~4.5us), so the K projection
                    # starts at full clock.
                    wps = pstile([128, 512], "pss", 5, "ps_wu")
                    for _ in range(40):
                        nc.tensor.matmul(wps[:, 0:128], wu_sb, wu_sb,
                                         start=True, stop=True)

                    # ---- K^T projection, ht-outer so matmuls chase the DMA.
                    kchunks = []
                    for mt in range(MT):
                        for po, pw in _chunks(KP, 512):
                            if pw > 128:
                                ps = pstile([128, 512], "pss", 5,
                                            f"ps_k{mt}_{po}")
                            else:
                                ps = pstile([128, 128], "pav", 2,
                                            f"ps_k{mt}_{po}")
                            kchunks.append((mt, po, pw, ps))
                    # the first two V-projection key-tiles ride along in
                    # the K ht-loop so the PE outpaces the xpT DMA stream
                    vps = [pstile([128, QD], "pss", 5, "ps_v0"),
                           pstile([128, QD], "pdn", 1, "ps_v1")]
                    for ht in range(HT):
                        for mt, po, pw, ps in kchunks:
                            nc.tensor.matmul(
                                ps[:, 0:pw],
                                wk_sb[:, ht, mt * 128:(mt + 1) * 128],
                                xpT_sb[:, ht, po:po + pw],
                                start=(ht == 0), stop=(ht == HT - 1))
                        for st in range(2):
                            nc.tensor.matmul(
                                vps[st],
                                xpT_sb[:, ht, st * 128:(st + 1) * 128],
                                wv_sb[:, ht, :],
                                start=(ht == 0), stop=(ht == HT - 1))
                    for mt, po, pw, ps in sorted(kchunks,
                                                 key=lambda c: -c[1]):
                        evac(kT_sb[:, mt, po:po + pw], ps[:, 0:pw])
                    for st in range(2):
                        evac(va_sb[:, st, :, 0:64],
                             vps[st].rearrange("p (h d) -> p h d", h=NHL))

                    # ---- V projection, remaining key-tiles (kt2/kt3
                    # ride the early-released pss/pdn slots)
                    for st in range(2, KT):
                        if st == 2:
                            pv = pstile([128, QD], "pss", 5, "ps_v2")
                        elif st == 3:
                            pv = pstile([128, QD], "pdn", 1, "ps_v3")
                        else:
                            pv = pstile([128, QD], "pav", 2, "ps_v")
                        for ht in range(HT):
                            nc.tensor.matmul(
                                pv, xpT_sb[:, ht, st * 128:(st + 1) * 128],
                                wv_sb[:, ht, :],
                                start=(ht == 0), stop=(ht == HT - 1))
                        evac(va_sb[:, st, :, 0:64],
                             pv.rearrange("p (h d) -> p h d", h=NHL))

                # ---- Q^T projection for a query half
                def emit_q(po, mts=(0, 1)):
                    for mt in mts:
                        for co, cw in _chunks(1024, 512):
                            ps = pstile([128, 512], "pss", 5,
                                        f"ps_q{mt}_{po + co}")
                            for ht in range(HT):
                                nc.tensor.matmul(
                                    ps,
                                    wq_sb[:, ht, mt * 128:(mt + 1) * 128],
                                    xT_sb[:, ht, po + co:po + co + cw],
                                    start=(ht == 0), stop=(ht == HT - 1))
                            evac(qT_sb[:, mt, po + co:po + co + cw], ps)

                emit_q(0)

                # ---- attention + interleaved out-projection phases
                with tile.TileContext.tile_pool(tc, name="pex", bufs=12) as pxp, \
                     tile.TileContext.tile_pool(tc, name="an", bufs=3) as anp, \
                     tile.TileContext.tile_pool(tc, name="rc", bufs=4) as rcp, \
                     tile.TileContext.tile_pool(tc, name="stg", bufs=10) as sgp:

                    def attn_pair(pair, po):
                        """Attention for heads `pair` on queries
                        [po*1024, (po+1)*1024)."""
                        mtq = pair[0] // 2
                        q0 = po * 1024
                        pav = {}
                        pend = {h: [] for h in pair}
                        for h in pair:
                            pav[h] = pstile([128, QT, 64], "pav", 2,
                                            f"pav{h}_{po}")
                        # softmax denominators for both heads (64B bank)
                        pden = pstile([128, 2, QT], "pdn", 1,
                                      f"pdn{pair[0]}_{po}")

                        def flush_av(hi, h):
                            # start=True zeroes the whole 2KB PSUM bank, so
                            # only the very first matmul into each bank may
                            # carry it; the other interleaved accumulation
                            # groups land on pending-zero bytes (zeroed on
                            # first write).
                            pkt, ppx = pend[h].pop(0)
                            for qt in range(QT):
                                pxs = ppx[:, qt * 128:(qt + 1) * 128]
                                nc.tensor.matmul(
                                    pav[h][:, qt, :], pxs,
                                    va_sb[:, pkt, h, 0:64],
                                    start=(pkt == 0 and qt == 0),
                                    stop=(pkt == KT - 1 and qt == QT - 1),
                                    skip_group_check=True)
                                nc.tensor.matmul(
                                    pden[:, hi, qt:qt + 1], pxs,
                                    va_sb[:, pkt, h, 64:65],
                                    start=(pkt == 0 and qt == 0 and hi == 0),
                                    stop=(pkt == KT - 1 and qt == QT - 1
                                          and hi == 1),
                                    skip_group_check=True)

                        for kt in range(KT):
                            pxs = {}
                            for hi, h in enumerate(pair):
                                pxs[h] = pxp.tile([128, 1024], F16, tag="pex",
                                                  name=f"pex{h}_{po}_{kt}")
                                pend[h].append((kt, pxs[h]))
                            for ci, (co, cw) in enumerate(_chunks(1024, 512)):
                                for hi, h in enumerate(pair):
                                    rb = (h * HD) % 128
                                    px = pxs[h]
                                    engs = _exp_engines(kt, hi,
                                                        last=(kt == KT - 1))
                                    ps = pstile([128, 512], "pss", 5,
                                                f"ps_s{h}_{po}_{kt}_{co}")
                                    nc.tensor.matmul(
                                        ps,
                                        kT_sb[rb:rb + HD, mtq,
                                              kt * 128:(kt + 1) * 128],
                                        qT_sb[rb:rb + HD, mtq,
                                              q0 + co:q0 + co + cw],
                                        start=True, stop=True)
                                    eng = engs[ci]
                                    if eng == "a":
                                        nc.scalar.activation(
                                            out=px[:, co:co + cw],
                                            in_=ps,
                                            func=mybir.ActivationFunctionType.Exp,
                                            bias=bk_sb[:, kt:kt + 1], scale=1.0)
                                    else:
                                        nc.vector.tensor_scalar(
                                            px[:, co:co + cw].bitcast(U16),
                                            ps,
                                            AEXP, bk2_sb[:, kt:kt + 1],
                                            mybir.AluOpType.mult,
                                            mybir.AluOpType.add)
                            # drain the AV backlog harder near the end of the
                            # kt loop so normalize/transpose start promptly
                            max_pend = 2 if kt < KT - 2 else (KT - 1 - kt)
                            for hi, h in enumerate(pair):
                                while len(pend[h]) > max_pend:
                                    flush_av(hi, h)
                        an = anp.tile([128, QT, 128], F16, tag="an",
                                      name=f"an{pair[0]}_{po}")
                        rcb = {}
                        for hi, h in enumerate(pair):
                            while pend[h]:
                                flush_av(hi, h)
                            rc = rcp.tile([128, QT], F32, tag="rc",
                                          name=f"rc{h}_{po}")
                            nc.vector.reciprocal(rc, pden[:, hi, :])
                            r = rc.rearrange("p (q o) -> p q o", o=1)
                            rcb[h] = r.broadcast_to([128, QT, 64])
                        # normalize a_n[q, qt, d] = pav[q, qt, d] / den in
                        # qt-half groups so transposes can start early
                        for hi, h in enumerate(pair):
                            nc.vector.tensor_tensor(
                                out=an[:, :, hi * 64:hi * 64 + 64],
                                in0=pav[h][:, :, :], in1=rcb[h],
                                op=mybir.AluOpType.mult)

                        # transpose [q, d] -> [d, q].  The first pair rides
                        # the DMA XBAR (latency-tolerant: a full pair of
                        # compute follows); the last pair, which gates the
                        # out-projection, uses the PE array + tiny evacs.
                        # Emission is deferred to the returned closure so the
                        # caller can slot PE filler before the PE transposes.
                        tr_tile = []

                        def finish_half(hq):
                            qts = range(hq * (QT // 2), (hq + 1) * (QT // 2))
                            if mtq == 0:
                                for qt in qts:
                                    nc.sync.dma_start_transpose(
                                        aT_sb[:, mtq,
                                              q0 + qt * 128:q0 + (qt + 1) * 128],
                                        an[:, qt, :])
                                return
                            if not tr_tile:
                                tr_tile.append(pstile([128, QT, 128], "pav", 2,
                                                      f"tr{po}", dt=F16))
                            tr = tr_tile[0]
                            for qt in qts:
                                # sub-bank writes: only the first transpose
                                # into the tr bank may carry start=True
                                nc.tensor.matmul(
                                    tr[:, qt, :], an[:, qt, :], id_sb,
                                    is_transpose=True,
                                    start=(hq == 0 and qt == qts[0]),
                                    stop=(hq == 1 and qt == qts[-1]),
                                    skip_group_check=True)
                            engs = ((nc.scalar, nc.vector, nc.scalar,
                                     nc.vector) if hq == 0 else
                                    (nc.vector, nc.vector, nc.vector,
                                     nc.vector))
                            for qt in qts:
                                dst = aT_sb[:, mtq,
                                            q0 + qt * 128:q0 + (qt + 1) * 128]
                                copy_half(engs[qt % 4], dst, tr[:, qt, :])

                        def finish():
                            finish_half(0)
                            finish_half(1)

                        finish.half = finish_half
                        return finish

                    def copy_half(eng, dst, src):
                        if eng is nc.scalar:
                            eng.copy(dst, src)
                        else:
                            eng.tensor_copy(dst, src)

                    def out_proj(ho, jts, eoff=0):
                        """Out-projection rows jts, columns [ho*1024, +1024).
                        PSUM evacuation runs as 512-halves on two engines so
                        the pss slots free at PE pace."""
                        q0 = ho * 1024
                        for ji, jt in enumerate(jts):
                            stg = sgp.tile([128, 1024], F16, tag="stg",
                                           name="stage")
                            e01 = ((nc.scalar, nc.vector)
                                   if (ji + eoff) % 2 == 0
                                   else (nc.vector, nc.scalar))
                            for ci, (co, cw) in enumerate(_chunks(1024, 512)):
                                pf = pstile([128, 512], "pss", 5,
                                            f"ps_f{jt}_{ho}_{co}")
                                for mt in range(MT):
                                    nc.tensor.matmul(
                                        pf,
                                        wo_sb[:, mt, jt * 128:(jt + 1) * 128],
                                        aT_sb[:, mt, q0 + co:q0 + co + cw],
                                        start=(mt == 0), stop=(mt == MT - 1))
                                copy_half(e01[ci], stg[:, co:co + cw], pf)
                            nc.sync.dma_start(
                                out=outT.ap()[jt * 128:(jt + 1) * 128,
                                              q0:q0 + 1024],
                                in_=stg)

                    attn_pair((0, 1), 0)()
                    f2 = attn_pair((2, 3), 0)
                    emit_q(1024, mts=(0,))   # PE filler while normalize lands
                    f2()
                    emit_q(1024, mts=(1,))
                    out_proj(0, range(5))
                    attn_pair((0, 1), 1)()
                    f4 = attn_pair((2, 3), 1)
                    # held-back ho=0 rows fill the PE while the last pair's
                    # normalize lands, then its PE transposes + evacs run
                    out_proj(0, range(5, 6), eoff=1)
                    f4.half(0)
                    out_proj(0, range(6, HT), eoff=1)
                    f4.half(1)
                    out_proj(1, range(HT))

    nc.compile()
    return nc


def _prep_inputs(hidden_states, attention_mask, w_qkv, w_out):
    """Shard + transpose + quantize inputs for the 8 cores."""
    hs = np.asarray(hidden_states, dtype=np.float32)
    mask = np.asarray(attention_mask)
    wqkv = np.asarray(w_qkv, dtype=np.float32)
    wo = np.asarray(w_out, dtype=np.float32)

    idxs = [np.nonzero(mask[b] != 0)[0] for b in range(B)]
    counts = [len(ix) for ix in idxs]
    KP = max(128, ((max(counts) + 127) // 128) * 128)
    KT = KP // 128

    xTs, xpTs, bks, bk2s = [], [], [], []
    for b in range(B):
        xb = hs[b].astype(NPF16)
        xTs.append(np.ascontiguousarray(xb.T))
        xp = np.zeros((KP, H), dtype=NPF16)
        xp[:counts[b]] = xb[idxs[b]]
        xpTs.append(np.ascontiguousarray(xp.T))
        bias = np.zeros(KP, dtype=np.float32)
        bias[counts[b]:] = -30000.0
        bias = np.ascontiguousarray(bias.reshape(KT, 128).T)
        bks.append(bias)
        bk2s.append(np.ascontiguousarray(
            (bias * AEXP + BEXP).astype(np.float32)))

    ident = np.ascontiguousarray(np.eye(128, dtype=NPF16))
    in_maps = []
    for c in range(NCORES):
        b, hb = c // CPB, c % CPB
        sl = slice(hb * QD, (hb + 1) * QD)
        in_maps.append({
            "ident": ident,
            "xT": xTs[b],
            "xpT": xpTs[b],
            "wqT": np.ascontiguousarray(
                (wqkv[sl, :] * SCALE).astype(NPF16).T),
            "wkT": np.ascontiguousarray(
                wqkv[H + sl.start:H + sl.stop, :].astype(NPF16).T),
            "wvT": np.ascontiguousarray(
                wqkv[2 * H + sl.start:2 * H + sl.stop, :].astype(NPF16).T),
            "woT": np.ascontiguousarray(wo[:, sl].astype(NPF16).T),
            "bk": bks[b],
            "bk2": bk2s[b],
        })
    return KP, in_maps


_NC_CACHE = {}


def kernel(hidden_states, attention_mask, w_qkv, w_out):
    KP, in_maps = _prep_inputs(hidden_states, attention_mask, w_qkv, w_out)
    if KP not in _NC_CACHE:
        _NC_CACHE[KP] = build_kernel(KP)
    nc = _NC_CACHE[KP]
    res = bass_utils.run_bass_kernel_spmd(nc, in_maps,
                                          core_ids=list(range(NCORES)))
    out = np.empty((B, S, H), dtype=np.float32)
    for b in range(B):
        acc = res.results[b * CPB]["outT"].astype(np.float32).copy()
        for c in range(b * CPB + 1, (b + 1) * CPB):
            acc += res.results[c]["outT"]
        out[b] = acc.T
    return out
